# revision 2
# baseline (speedup 1.0000x reference)
"""Trainium2 Bass kernel for nn_MultiHeadAttention_41944650612760.

Wasserstein-distance multi-head attention with cumulative position decay.
Sharding: data-parallel over batch B=8 across 8 NeuronCores (one batch/core).

Per-core pipeline (T=1024, D=512, H=8, dk=64), in [t-part, s-free] layout:
  P1  six linear projections on PE; q/k score operands kept transposed
      ([dout, t]) and head-stacked (parity split) so the score matmul
      contracts K=128; v projections kept normal ([s, dout]) in fp16.
  P2  row/col norm terms a1/b1 via ones-matmuls into [8, T] PSUM rows,
      bounced through a DRAM scratch so they come back as [1, T] rows
      for K=1 augment matmuls.
  P3  per (head, row-block): scores matmul -> PSUM, causal mask add,
      exp (ACT, fused 0.125 scale), cumsum scan (DVE tensor_tensor_scan),
      tail = (sm1-C)*|t-s| (Pool stt), sqrt+exp decay (ACT), second
      softmax numerator, normalize -> fp16, DMA-xbar transpose, PV matmuls.
  P4  fp16 output projections with bias via K=1 ones-augment.

zero_pad is applied on the host (row 0 of each output = bias).
"""

import os
import numpy as np
from contextlib import ExitStack

B, T, D, H = 8, 1024, 512, 8
DK = D // H          # 64
NT = T // 128        # 8 row/col blocks
NEG = -1e30
F16 = np.float16

# packed causal layout for transposed attention weights: block j holds
# t in [j*128, 1024) -> width (8-j)*128, at column offset OFF[j]
OFF = [0] * NT
for _j in range(1, NT):
    OFF[_j] = OFF[_j - 1] + (NT - (_j - 1)) * 128
P2T_COLS = OFF[-1] + 128  # 4608
# packed row-block layout: block tb holds s in [0, (tb+1)*128) at NOFF[tb]
NOFF = [0] * NT
for _t in range(1, NT):
    NOFF[_t] = NOFF[_t - 1] + _t * 128
NN_COLS = NOFF[-1] + NT * 128  # 4608


def _build(gamma2):
    """Trace the Bass program. gamma2[h] = gamma_h**2 (trace-time floats)."""
    import concourse.bass as bass
    import concourse.bacc as bacc
    import concourse.mybir as mybir
    import concourse.tile as tile

    dt = mybir.dt
    AF = mybir.ActivationFunctionType
    OP = mybir.AluOpType
    ts = bass.ts

    nc = bacc.Bacc()

    # ---- per-core DRAM I/O ----
    xT = nc.declare_dram_parameter("xT", [6, D, T], dt.float32, isOutput=False)
    wT = nc.declare_dram_parameter("wT", [4, D, D], dt.float32, isOutput=False)
    woT = nc.declare_dram_parameter("woT", [2, D, D], dt.float16, isOutput=False)
    wc = nc.declare_dram_parameter("wc", [D, H], dt.float32, isOutput=False)
    bqk = nc.declare_dram_parameter("bqk", [128, 12], dt.float32, isOutput=False)
    bvp = nc.declare_dram_parameter("bvp", [128, 8], dt.float32, isOutput=False)
    bvo16 = nc.declare_dram_parameter("bvo16", [2, D], dt.float16, isOutput=False)
    cb = nc.declare_dram_parameter("cb", [8, 2], dt.float32, isOutput=False)
    cbr = nc.declare_dram_parameter("cbr", [1, 8], dt.float32, isOutput=False)
    btri = nc.declare_dram_parameter("btri", [128, 128], dt.float32, isOutput=False)
    nneg = nc.declare_dram_parameter("nneg", [128, NN_COLS], dt.float16, isOutput=False)
    out_m = nc.declare_dram_parameter("out_m", [T, D], dt.float32, isOutput=True)
    out_c = nc.declare_dram_parameter("out_c", [T, D], dt.float32, isOutput=True)

    with tile.TileContext(nc) as tc, ExitStack() as ctx:
        pc = ctx.enter_context(tc.tile_pool(name="pc", bufs=1))
        pdr = ctx.enter_context(tc.tile_pool(name="pdr", bufs=1, space="DRAM"))

        # ---- persistent SBUF tensors ----
        U2 = pc.tile([128, H, T], dt.float32)       # [qm_h ; sqq_h] per head (parity split)
        W2 = pc.tile([128, H, T], dt.float32)       # [2km_h ; 2sqk_h]
        vm16 = pc.tile([128, NT, D], dt.float16)    # vm normal layout fp16
        vc16 = pc.tile([128, NT, D], dt.float16)
        nneg_sb = pc.tile([128, NN_COLS], dt.float16)
        btri_sb = pc.tile([128, 128], dt.float32)
        ones = pc.tile([1, T], dt.float32)
        ones16 = pc.tile([1, T], dt.float16)
        a1n8 = pc.tile([128, NT, 8], dt.float32)    # -0.125 * a1[t] per (tb, h)
        cbn8 = pc.tile([128, 8], dt.float32)        # -0.125 * cbias, bcast to 128 parts
        bqk_sb = pc.tile([128, 12], dt.float32)     # [bk|bkc_sw|2bk] x 4 chunks
        bvp_sb = pc.tile([128, 8], dt.float32)      # bv, bvc pair-sliced
        cb_sb = pc.tile([8, 2], dt.float32)
        wc_sb = pc.tile([128, 4, H], dt.float32)
        E2q = pc.tile([128, 4, 8], dt.float32)
        E2k = pc.tile([128, 4, 8], dt.float32)

        b_dr = pdr.tile([H, T], dt.float32)         # -b1 rows via DRAM bounce
        cm_dr = pdr.tile([2, D, T], dt.float16)     # attention out via DRAM bounce

        nc.sync.dma_start(out=nneg_sb, in_=nneg[:, :])
        nc.sync.dma_start(out=btri_sb, in_=btri[:, :])
        nc.sync.dma_start(out=bqk_sb, in_=bqk[:, :])
        nc.sync.dma_start(out=bvp_sb, in_=bvp[:, :])
        nc.sync.dma_start(out=cb_sb, in_=cb[:, :])
        nc.sync.dma_start(out=wc_sb, in_=wc.rearrange("(k p) h -> p k h", p=128))
        nc.vector.memset(ones, 1.0)
        nc.vector.memset(ones16, 1.0)
        _cbr = cbr[0:1, :]
        nc.sync.dma_start(out=cbn8, in_=bass.AP(tensor=_cbr.tensor, offset=_cbr.offset,
                                                ap=[[0, 128]] + list(_cbr.ap[1:])))
        nc.vector.memset(E2q, 0.0)
        nc.vector.memset(E2k, 0.0)
        for c in range(4):
            nc.vector.memset(E2q[0:64, c, 2 * c:2 * c + 1], 1.0)
            nc.vector.memset(E2q[64:128, c, 2 * c + 1:2 * c + 2], 1.0)
            nc.vector.memset(E2k[0:64, c, 2 * c:2 * c + 1], 0.25)
            nc.vector.memset(E2k[64:128, c, 2 * c + 1:2 * c + 2], 0.25)

        # =================== P1: projections + P2: a1/b1 ===================
        with tc.tile_pool(name="p1x", bufs=6) as px, \
             tc.tile_pool(name="p1w", bufs=1) as pw, \
             tc.tile_pool(name="p1z", bufs=3) as pz, \
             tc.tile_pool(name="p1r", bufs=1) as pr1, \
             tc.tile_pool(name="p1ps", bufs=2, space="PSUM") as pps, \
             tc.tile_pool(name="p1pa", bufs=1, space="PSUM") as ppa:

            b_ps = ppa.tile([8, T], dt.float32, tag="b_ps")   # b1 = m2sq + kcs
            a1t_ps = ppa.tile([128, NT, 8], dt.float32, tag="a1t")  # a1 in [t, (tb,h)]

            def load_x(i):
                xs = []
                for k in range(4):
                    xt = px.tile([128, T], dt.float32, tag="xt")
                    nc.sync.dma_start(out=xt, in_=xT[i, ts(k, 128), :])
                    xs.append(xt)
                return xs

            def load_w(i, tagslot):
                wt = pw.tile([128, 4, D], dt.float32, tag=f"wt{tagslot}")
                nc.sync.dma_start(out=wt, in_=wT[i].rearrange("(k p) d -> p k d", p=128))
                return wt

            # ---- run 1: qm -> U2 (bias bk) ----
            xq = load_x(0)
            wk = load_w(0, 0)
            wkc = load_w(1, 1)
            for c in range(4):
                for n in range(2):
                    ps = pps.tile([128, 512], dt.float32, tag="ps")
                    for k in range(4):
                        nc.tensor.matmul(ps, wk[:, k, ts(c, 128)],
                                         xq[k][:, ts(n, 512)],
                                         start=(k == 0), stop=(k == 3))
                    nc.scalar.activation(out=U2[0:64, 2 * c, ts(n, 512)], in_=ps[0:64],
                                         func=AF.Identity, bias=bqk_sb[0:64, c:c + 1])
                    nc.scalar.activation(out=U2[64:128, 2 * c + 1, ts(n, 512)], in_=ps[64:128],
                                         func=AF.Identity, bias=bqk_sb[64:128, c:c + 1])

            # ---- run 2: sqq -> U2 (clip+sqrt) + qcs into a_ps ----
            xqc = load_x(1)
            for c in range(4):
                for n in range(2):
                    ps = pps.tile([128, 512], dt.float32, tag="ps")
                    for k in range(4):
                        nc.tensor.matmul(ps, wkc[:, k, ts(c, 128)],
                                         xqc[k][:, ts(n, 512)],
                                         start=(k == 0), stop=(k == 3))
                    nc.vector.tensor_scalar(out=ps, in0=ps, scalar1=bqk_sb[:, 4 + c:5 + c],
                                            scalar2=1e-24, op0=OP.add, op1=OP.max)
                    # swapped col order: psum[0:64] = head 2c+1 (odd -> low parts)
                    nc.scalar.activation(out=U2[0:64, 2 * c + 1, ts(n, 512)], in_=ps[0:64],
                                         func=AF.Sqrt)
                    nc.scalar.activation(out=U2[64:128, 2 * c, ts(n, 512)], in_=ps[64:128],
                                         func=AF.Sqrt)

            # ---- q-side squares + m1sq + qcs matmuls into a1T form ----
            # one full accumulation group per tb (bank-level group tracking)
            zqs = []
            for c in range(4):
                zq = pz.tile([128, T], dt.float32, tag=f"zq{c}", name="zq", bufs=1)
                nc.scalar.activation(out=zq[0:64, :], in_=U2[0:64, 2 * c, :], func=AF.Square)
                nc.scalar.activation(out=zq[64:128, :], in_=U2[64:128, 2 * c + 1, :], func=AF.Square)
                zqs.append(zq)
            for tb in range(NT):
                for k in range(4):
                    nc.tensor.matmul(a1t_ps[:, tb, :], xqc[k][:, ts(tb, 128)],
                                     wc_sb[:, k, :],
                                     start=(k == 0), stop=False)
                for c in range(4):
                    nc.tensor.matmul(a1t_ps[:, tb, :], zqs[c][:, ts(tb, 128)],
                                     E2q[:, c, :],
                                     start=False, stop=(c == 3))
            nc.scalar.activation(out=a1n8, in_=a1t_ps, func=AF.Copy, scale=-0.125)
            nc.vector.tensor_tensor(out=a1n8, in0=a1n8,
                                    in1=bass.AP(tensor=cbn8.tensor, offset=cbn8.offset,
                                                ap=[cbn8.ap[0], [0, NT]] + list(cbn8.ap[1:])),
                                    op=OP.add)

            # ---- run 3: 2km -> W2 (bias 2bk, scale 2) ----
            xk = load_x(2)
            for c in range(4):
                for n in range(2):
                    ps = pps.tile([128, 512], dt.float32, tag="ps")
                    for k in range(4):
                        nc.tensor.matmul(ps, wk[:, k, ts(c, 128)],
                                         xk[k][:, ts(n, 512)],
                                         start=(k == 0), stop=(k == 3))
                    nc.scalar.activation(out=W2[0:64, 2 * c, ts(n, 512)], in_=ps[0:64],
                                         func=AF.Identity, scale=2.0, bias=bqk_sb[0:64, 8 + c:9 + c])
                    nc.scalar.activation(out=W2[64:128, 2 * c + 1, ts(n, 512)], in_=ps[64:128],
                                         func=AF.Identity, scale=2.0, bias=bqk_sb[64:128, 8 + c:9 + c])

            # ---- run 4: 2sqk -> W2 + kcs into b_ps ----
            xkc = load_x(3)
            for n in range(2):
                for k in range(4):
                    nc.tensor.matmul(b_ps[:, ts(n, 512)], wc_sb[:, k, :],
                                     xkc[k][:, ts(n, 512)],
                                     start=(k == 0), stop=False)
            for c in range(4):
                for n in range(2):
                    ps = pps.tile([128, 512], dt.float32, tag="ps")
                    for k in range(4):
                        nc.tensor.matmul(ps, wkc[:, k, ts(c, 128)],
                                         xkc[k][:, ts(n, 512)],
                                         start=(k == 0), stop=(k == 3))
                    nc.vector.tensor_scalar(out=ps, in0=ps, scalar1=bqk_sb[:, 4 + c:5 + c],
                                            scalar2=1e-24, op0=OP.add, op1=OP.max)
                    nc.scalar.activation(out=W2[0:64, 2 * c + 1, ts(n, 512)], in_=ps[0:64],
                                         func=AF.Sqrt, scale=4.0)
                    nc.scalar.activation(out=W2[64:128, 2 * c, ts(n, 512)], in_=ps[64:128],
                                         func=AF.Sqrt, scale=4.0)

            # ---- k-side squares (of 2km; E2k carries the 1/4) ----
            for c in range(4):
                zk = pz.tile([128, T], dt.float32, tag="z", bufs=2)
                nc.vector.tensor_mul(zk[0:64, :], W2[0:64, 2 * c, :], W2[0:64, 2 * c, :])
                nc.vector.tensor_mul(zk[64:128, :], W2[64:128, 2 * c + 1, :], W2[64:128, 2 * c + 1, :])
                for n in range(2):
                    nc.tensor.matmul(b_ps[:, ts(n, 512)], E2k[:, c, :],
                                     zk[:, ts(n, 512)],
                                     start=False, stop=(c == 3))
            stg_b = pz.tile([8, T], dt.float32, tag="stg", bufs=1)
            nc.scalar.activation(out=stg_b, in_=b_ps, func=AF.Identity,
                                 scale=-1.0, bias=cb_sb[:, 1:2])
            nc.sync.dma_start(out=b_dr[:], in_=stg_b)

            # ---- runs 5/6: vm, vc (normal layout, fp16) ----
            for i, (xi, wi, dest) in enumerate([(4, 2, vm16), (5, 3, vc16)]):
                xv = load_x(xi)
                wv = load_w(wi, i % 2)
                for m in range(NT):
                    ps = pps.tile([128, 512], dt.float32, tag="ps")
                    for k in range(4):
                        nc.tensor.matmul(ps, xv[k][:, ts(m, 128)],
                                         wv[:, k, :],
                                         start=(k == 0), stop=(k == 3))
                    nc.scalar.activation(out=dest[:, m, :], in_=ps, func=AF.Copy)

        # =================== P3: attention ===================
        with tc.tile_pool(name="wkp", bufs=2) as pwk, \
             tc.tile_pool(name="scp", bufs=5) as psc, \
             tc.tile_pool(name="dgp", bufs=4) as pdg, \
             tc.tile_pool(name="p16", bufs=2) as p16, \
             tc.tile_pool(name="pt", bufs=1) as pt, \
             tc.tile_pool(name="prow", bufs=1) as prow, \
             tc.tile_pool(name="stgo", bufs=2) as pstg, \
             tc.tile_pool(name="tiny", bufs=8) as ptiny, \
             tc.tile_pool(name="ps_s", bufs=2, space="PSUM") as pps_s, \
             tc.tile_pool(name="ps_o", bufs=1, space="PSUM") as pps_o:

            om_m = om_c = None
            for h in range(H):
                g2 = float(gamma2[h])
                b1rown = prow.tile([1, T], dt.float32, tag="b1rown")
                nc.sync.dma_start(out=b1rown, in_=b_dr[h:h + 1, :])
                b1b = prow.tile([128, T], dt.float32, tag="b1b")
                nc.gpsimd.partition_broadcast(b1b, b1rown)
                p2T = pt.tile([128, NT, T], dt.float16, tag="p2T")
                for half in range(2):
                    tbs = list(range(half * 4, half * 4 + 4))
                    sc = {}
                    dg = {}
                    r1 = {}
                    gn = {}
                    for tb in tbs:
                        W = (tb + 1) * 128
                        nchunks = [(0, min(W, 512))] + ([(512, W)] if W > 512 else [])
                        ps = pps_s.tile([128, 1024], dt.float32, tag="ps_s")
                        for (s0, s1) in nchunks:
                            nc.tensor.matmul(ps[:, s0:s1], U2[:, h, ts(tb, 128)],
                                             W2[:, h, s0:s1], start=True, stop=True)
                        nc.vector.tensor_tensor(out=ps[:, :W], in0=ps[:, :W],
                                                in1=b1b[:, :W], op=OP.add)
                        nc.vector.tensor_tensor(out=ps[:, tb * 128:W], in0=ps[:, tb * 128:W],
                                                in1=btri_sb, op=OP.add)
                        # sc = 0.125*psum - 0.125*a1[t] = true scores (table-free evict)
                        sc[tb] = psc.tile([128, T], dt.float32, tag="sc", name="sc")
                        nc.scalar.activation(out=sc[tb][:, :W], in_=ps[:, :W], func=AF.Identity,
                                             scale=0.125, bias=a1n8[:, tb, h:h + 1])
                    for tb in tbs:
                        W = (tb + 1) * 128
                        e = pwk.tile([128, T], dt.float32, tag="e")
                        nc.scalar.activation(out=e[:, :W], in_=sc[tb][:, :W], func=AF.Exp)
                        C = pwk.tile([128, T], dt.float32, tag="C")
                        nc.vector.tensor_tensor_scan(out=C[:, :W], data0=e[:, :W], data1=e[:, :W],
                                                     initial=0.0, op0=OP.add, op1=OP.bypass)
                        sm1 = C[:, W - 1:W]
                        r1[tb] = ptiny.tile([128, 1], dt.float32, tag="rcp1", name="rcp1")
                        nc.vector.reciprocal(out=r1[tb], in_=sm1)
                        # bound2 = sm1 * (ln(1e-5)^2/gamma^2): min(darg, bound2) caps the
                        # decay exponent at ln(1e-5) => te >= 1e-5 (the reference clip)
                        bnd = ptiny.tile([128, 1], dt.float32, tag="bnd")
                        nc.vector.tensor_scalar(out=bnd, in0=sm1, scalar1=132.54668 / g2,
                                                scalar2=None, op0=OP.mult)
                        dg[tb] = pdg.tile([128, T], dt.float32, tag="darg", name="darg")
                        nc.vector.scalar_tensor_tensor(out=dg[tb][:, :W], in0=C[:, :W], scalar=sm1,
                                                       in1=nneg_sb[:, NOFF[tb]:NOFF[tb] + W],
                                                       op0=OP.subtract, op1=OP.mult)
                        nc.vector.tensor_scalar(out=dg[tb][:, :W], in0=dg[tb][:, :W],
                                                scalar1=bnd, scalar2=None, op0=OP.min)
                    for tb in tbs:
                        W = (tb + 1) * 128
                        nc.scalar.activation(out=dg[tb][:, :W], in_=dg[tb][:, :W], func=AF.Sqrt)
                        gs = ptiny.tile([128, 1], dt.float32, tag="gs")
                        nc.scalar.activation(out=gs, in_=r1[tb], func=AF.Sqrt, scale=g2)
                        gn[tb] = ptiny.tile([128, 1], dt.float32, tag="gsn", name="gsn")
                        nc.vector.tensor_scalar(out=gn[tb], in0=gs, scalar1=-1.0, scalar2=None,
                                                op0=OP.mult)
                    for tb in tbs:
                        W = (tb + 1) * 128
                        te = pwk.tile([128, T], dt.float32, tag="te")
                        nc.scalar.activation(out=te[:, :W], in_=dg[tb][:, :W], func=AF.Exp,
                                             scale=gn[tb])
                        nc.gpsimd.tensor_tensor(out=te[:, :W], in0=te[:, :W],
                                                in1=sc[tb][:, :W], op=OP.mult)
                        sm2 = ptiny.tile([128, 1], dt.float32, tag="sm2")
                        nc.scalar.activation(out=te[:, :W], in_=te[:, :W], func=AF.Exp,
                                             accum_out=sm2)
                        rcp2 = ptiny.tile([128, 1], dt.float32, tag="rcp2")
                        nc.vector.reciprocal(out=rcp2, in_=sm2)
                        p2 = p16.tile([128, T], dt.float16, tag="p2")
                        nc.vector.tensor_scalar(out=p2[:, :W], in0=te[:, :W], scalar1=rcp2,
                                                scalar2=None, op0=OP.mult)
                        nc.sync.dma_start_transpose(out=p2T[:, 0:tb + 1, ts(tb, 128)],
                                                    in_=p2[:, :W])

                # ---- PV for head h (pair-shared psum) ----
                half_p = (h % 2) * 64
                if h % 2 == 0:
                    om_m = pps_o.tile([128, 1024], dt.float32, tag="om_m")
                    om_c = pps_o.tile([128, 1024], dt.float32, tag="om_c")
                hs = slice(h * DK, (h + 1) * DK)
                for j in range(NT):
                    w_j = (NT - j) * 128
                    tr = [(j * 128, 512), (512, 1024)] if j < 4 else [(j * 128, 1024)]
                    def _stop(t0, t1, j=j):
                        return (j == 3) if t1 <= 512 else (j == 7)
                    for (t0, t1) in tr:
                        nc.tensor.matmul(om_m[half_p:half_p + 64, t0:t1], vm16[:, j, hs],
                                         p2T[:, j, t0:t1],
                                         start=(j == 0), stop=_stop(t0, t1))
                    p2sq = p16.tile([128, T], dt.float16, tag="p2sq")
                    nc.gpsimd.tensor_mul(p2sq[:, :w_j], p2T[:, j, j * 128:1024],
                                         p2T[:, j, j * 128:1024])
                    for (t0, t1) in tr:
                        nc.tensor.matmul(om_c[half_p:half_p + 64, t0:t1], vc16[:, j, hs],
                                         p2sq[:, t0 - j * 128: t1 - j * 128],
                                         start=(j == 0), stop=_stop(t0, t1))
                if h % 2 == 1:
                    pair = h // 2
                    st_m = pstg.tile([128, T], dt.float16, tag="st_m")
                    nc.scalar.activation(out=st_m, in_=om_m, func=AF.Identity,
                                         bias=bvp_sb[:, pair:pair + 1])
                    nc.sync.dma_start(out=cm_dr[0, 128 * pair:128 * (pair + 1), :], in_=st_m)
                    st_c = pstg.tile([128, T], dt.float16, tag="st_c")
                    nc.vector.tensor_scalar(out=st_c, in0=om_c, scalar1=bvp_sb[:, 4 + pair:5 + pair],
                                            scalar2=None, op0=OP.add)
                    nc.sync.dma_start(out=cm_dr[1, 128 * pair:128 * (pair + 1), :], in_=st_c)

        # =================== P4: output projections ===================
        with tc.tile_pool(name="p4w", bufs=2) as p4w, \
             tc.tile_pool(name="p4c", bufs=2) as p4c, \
             tc.tile_pool(name="p4s", bufs=2) as p4s, \
             tc.tile_pool(name="p4r", bufs=1) as p4r, \
             tc.tile_pool(name="p4ps", bufs=2, space="PSUM") as p4ps:
            for i, dst in enumerate([out_m, out_c]):
                wo = p4w.tile([128, 4, D], dt.float16, tag="wo")
                nc.sync.dma_start(out=wo, in_=woT[i].rearrange("(k p) d -> p k d", p=128))
                cmt = p4c.tile([128, 4, T], dt.float16, tag="cmt")
                nc.sync.dma_start(out=cmt, in_=cm_dr[i].rearrange("(k p) t -> p k t", p=128))
                borow = p4r.tile([1, D], dt.float16, tag="borow")
                nc.sync.dma_start(out=borow, in_=bvo16[i:i + 1, :])
                for m in range(NT):
                    ps = p4ps.tile([128, 512], dt.float32, tag="ps4")
                    for k in range(4):
                        nc.tensor.matmul(ps, cmt[:, k, ts(m, 128)], wo[:, k, :],
                                         start=(k == 0), stop=False)
                    nc.tensor.matmul(ps, ones16[0:1, ts(m, 128)], borow,
                                     start=False, stop=True)
                    st = p4s.tile([128, 512], dt.float32, tag="st4")
                    nc.scalar.activation(out=st, in_=ps, func=AF.Copy)
                    nc.sync.dma_start(out=dst[ts(m, 128), :], in_=st)

    nc.finalize()
    return nc


def kernel(**inputs):
    f32 = lambda k: np.ascontiguousarray(np.asarray(inputs[k], np.float32))
    Wk, bk = f32('Wk_mean'), f32('bk_mean')
    Wkc, bkc = f32('Wk_cov'), f32('bk_cov')
    Wv, bv = f32('Wv_mean'), f32('bv_mean')
    Wvc, bvc = f32('Wv_cov'), f32('bv_cov')
    Wo, bo = f32('Wo_mean'), f32('bo_mean')
    Woc, boc = f32('Wo_cov'), f32('bo_cov')
    gammas = f32('gammas').reshape(H)
    zero_pad = int(np.asarray(inputs['zero_pad']))

    gamma = -np.log1p(np.exp(gammas))          # -softplus
    gamma2 = (gamma * gamma).astype(np.float64)

    # head-pair-swapped column permutation for the cov-side weights
    perm = np.arange(D).reshape(4, 2, DK)[:, ::-1, :].reshape(D)
    WkcT_sw = np.ascontiguousarray(Wkc.T[:, perm])
    bkc_sw = bkc[perm]

    wT = np.stack([np.ascontiguousarray(Wk.T), WkcT_sw,
                   np.ascontiguousarray(Wv.T), np.ascontiguousarray(Wvc.T)])
    woT = np.stack([np.ascontiguousarray(Wo.T), np.ascontiguousarray(Woc.T)]).astype(F16)
    wc = np.ascontiguousarray(Wkc.T.reshape(D, H, DK).sum(-1))   # [din, H]

    bqk = np.zeros((128, 12), np.float32)
    bqk[:, 0:4] = bk.reshape(4, 128).T
    bqk[:, 4:8] = bkc_sw.reshape(4, 128).T
    bqk[:, 8:12] = 2.0 * bk.reshape(4, 128).T
    bvo16 = np.stack([bo, boc]).astype(F16)
    bvp = np.concatenate([bv.reshape(4, 128).T, bvc.reshape(4, 128).T], axis=1).astype(np.float32)
    sb = bkc.reshape(H, DK).sum(-1)
    cbt = np.stack([sb, -sb], axis=1).astype(np.float32)         # [8, 2]
    cbr = np.ascontiguousarray((-0.125 * sb)[None, :]).astype(np.float32)  # [1, 8]

    btri = np.triu(np.full((128, 128), NEG, np.float32), 1)
    idx_t = np.arange(T)
    nneg = np.zeros((128, NN_COLS), np.float32)
    for tb in range(NT):
        tt = tb * 128 + np.arange(128)
        W = (tb + 1) * 128
        nneg[:, NOFF[tb]:NOFF[tb] + W] = -np.abs(tt[:, None] - idx_t[None, :W])
    nneg = nneg.astype(F16)

    xs = [f32('q_mean'), f32('q_cov'), f32('k_mean'), f32('k_cov'),
          f32('v_mean'), f32('v_cov')]

    nc = _build(gamma2)

    in_maps = []
    for b in range(B):
        xTb = np.stack([np.ascontiguousarray(x[b].T) for x in xs])
        in_maps.append(dict(xT=xTb, wT=wT, woT=woT, wc=wc, bqk=bqk, bvp=bvp,
                            bvo16=bvo16, cb=cbt, cbr=cbr, btri=btri, nneg=nneg))

    from concourse.bass_utils import run_bass_kernel_spmd
    trace = bool(int(os.environ.get("KERNEL_TRACE", "0")))
    kw = {}
    if os.environ.get("KERNEL_TMPDIR"):
        kw["tmpdir"] = os.environ["KERNEL_TMPDIR"]
    res = run_bass_kernel_spmd(nc, in_maps, list(range(B)), trace=trace, **kw)
    if trace and res.exec_time_ns is not None:
        print(f"HW exec time: {res.exec_time_ns} ns")
        if res.mean_exec_time_ns is not None:
            print(f"HW exec time mean: {res.mean_exec_time_ns:.0f} ns")

    out_mean = np.stack([res.results[b]["out_m"] for b in range(B)])
    out_cov = np.stack([res.results[b]["out_c"] for b in range(B)])
    if zero_pad:
        out_mean[:, 0, :] = bo[None, :]
        out_cov[:, 0, :] = boc[None, :]
    return out_mean, out_cov

